# revision 20
# baseline (speedup 1.0000x reference)
"""Trainium2 Bass kernel for BlockSample (masked 4x4 one-hot conv / im2col).

Semantics (per batch b):
  out[(b, r, c), k, i*4+j] = x[b, k, r+i-3, c+j-2]   (zero padded)
  with taps (i=3, j>=2) forced to zero.

Full input  : (8, 192, 48, 48) f32
Full output : (8*48*48, 192, 4, 4) f32

Sharding: pure data parallel over the batch dim -> 8 NeuronCores, one
batch image per core, no cross-core communication.

Per-core plan:
  - Load image k-major into SBUF: A[ch, FRONT + r*48 + c] with zero pads
    front/back so row shifts (i-3) fall into zeros. ch 128..191 are
    loaded twice (partitions 0..63 and 64..127) so the two K=64
    transposes of a tap pair can run in distinct PE row groups.
  - For each 128-pixel block and each live tap: PE transpose-mode matmul
    (data stationary, fp32 identity moving) -> PSUM [pix, ch].
  - DVE/ACT copy PSUM -> SBUF store tile with interleaved dest AP
    (col = ch*16 + tap), multiplying by a per-partition column-border
    mask (handles c+dj wraparound across rows).
  - Contiguous [128 pix, 12288 B] rows -> HBM stores (mostly 2-block,
    3 MB per DMA).
"""

import numpy as np

import concourse.bass as bass
import concourse.tile as tile
from concourse import bacc, mybir
from concourse import bass_utils

F32 = mybir.dt.float32

CH = 192          # channels
HW = 48           # image height/width
NPIX = HW * HW    # 2304 pixels per image
NT = 16           # taps (4x4)
NB = 8            # batch / cores
FRONT = 160       # front zero pad (>= 146 = 3*48+2)
BACK = 16         # back zero pad (>= 2)
AW = FRONT + NPIX + BACK   # padded image width in f32 elems (2480)
BLK = 128         # pixels per block
NBLK = NPIX // BLK  # 18

STORE_BLKS = 2    # blocks per store DMA (steady state)
NSLOTS = 4        # rotating store tiles
SPLIT64 = False   # split K=128 transposes into two row-group K=64 transposes
F32R = False      # run PE transposes as float32r (1.5 vs 2.0 cyc/row)

# taps: t = i*4 + j, pixel offset = (i-3)*48 + (j-2); (i=3, j>=2) are zero.
# unit (j, a) covers taps i = 2a + s for s < n_s; one PSUM tile per unit.
UNITS = []
for _j in range(4):
    for _a in range(2):
        _n = 1 if (_a == 1 and _j >= 2) else 2
        UNITS.append((_j, _a, _n))

# engine split for the PSUM->SBUF copies: DVE = 2 pairs + 2 singles, ACT = 4 pairs
_DVE_UNITS = {(0, 0), (0, 1), (2, 1), (3, 1)}


def _const_data():
    """[128, 137] = identity(128) ++ 9 border masks [dj in (-2,-1,1)] x [phase 0..2]."""
    ident = np.eye(128, dtype=np.float32)
    masks = np.ones((128, 9), dtype=np.float32)
    q = np.arange(128)
    for dji, dj in enumerate((-2, -1, 1)):
        for ph in range(3):
            phase = (BLK * ph) % HW  # 0, 32, 16
            c = (phase + q) % HW
            bad = (c + dj < 0) | (c + dj >= HW)
            masks[bad, dji * 3 + ph] = 0.0
    return np.concatenate([ident, masks], axis=1)


_MASK_COL = {-2: 0, -1: 3, 1: 6}


def _image_pass(nc, x, y, a1, a2, cst, slots, groups, pspool):
    """One full image -> output pass (loads, transposes, copies, stores)."""
    ident = cst[:, 0:128]

    def mask_ap(dj, bl):
        return cst[:, 128 + _MASK_COL[dj] + (bl % 3):129 + _MASK_COL[dj] + (bl % 3)]

    # chunked loads so block-0 compute can start early; a2 loads go on the
    # ACT HWDGE ring, a1 on the SP ring, so descriptor generation overlaps
    bounds = [0, 448, NPIX]
    for ci in range(len(bounds) - 1):
        lo, hi = bounds[ci], bounds[ci + 1]
        nc.sync.dma_start(out=a1[:, FRONT + lo:FRONT + hi],
                          in_=x.ap()[0:128, lo:hi])
        nc.scalar.dma_start(out=a2[0:64, FRONT + lo:FRONT + hi],
                            in_=x.ap()[128:192, lo:hi])
        nc.scalar.dma_start(out=a2[64:128, FRONT + lo:FRONT + hi],
                            in_=x.ap()[128:192, lo:hi])

    for g, blks in enumerate(groups):
        t_ = slots[g % NSLOTS]
        tv = t_.rearrange("p (b c a s j) -> p b c a s j",
                          b=STORE_BLKS, c=CH, a=2, s=2, j=4)
        for bsub, bl in enumerate(blks):
            # One PSUM tile (= one bank) per unit. Tile serializes every
            # same-bank op pair (the P10 PE-W/DVE-R hazard guard), so a
            # unit's 4 transposes + copy form a latency-exposed chain.
            # Emit transposes interleaved across groups of 4 units: the
            # same-bank gap is >= 4 instructions (drain latency hidden)
            # and the first group's copies overlap the second group's
            # matmuls. HW-measured 200 -> 125 us/image vs naive order.
            unit_ps = {}
            for u in range(len(UNITS)):
                unit_ps[u] = pspool.tile([128, 384], F32, tag="ps",
                                         name=f"ps_{bl}_{u}")

            def unit_mms(u):
                j, a, n_s = UNITS[u]
                ps = unit_ps[u]
                out = []
                for s in range(n_s):
                    i = 2 * a + s
                    w = FRONT + bl * BLK + (i - 3) * HW + (j - 2)
                    rb = 64 * s
                    if SPLIT64:
                        # all-K=64 transposes; alternating PE row groups
                        # allow array-level concurrency
                        out.append((ps[:, s * 192:s * 192 + 64],
                                    a1[0:64, w:w + 128],
                                    cst[0:64, 0:64], (0, 0)))
                        out.append((ps[:, s * 192 + 64:s * 192 + 128],
                                    a1[64:128, w:w + 128],
                                    cst[64:128, 64:128], (64, 0)))
                        out.append((ps[:, s * 192 + 128:s * 192 + 192],
                                    a2[rb:rb + 64, w:w + 128],
                                    cst[rb:rb + 64, rb:rb + 64], (rb, 0)))
                    else:
                        # K=128 chunk (ch 0..127)
                        out.append((ps[:, s * 192:s * 192 + 128],
                                    a1[:, w:w + 128], ident[0:128, 0:128],
                                    (0, 0)))
                        # K=64 chunk (ch 128..191); s=1 uses PE row group 64
                        # so a pair's K=64 transposes can overlap in the array
                        out.append((ps[:, s * 192 + 128:s * 192 + 192],
                                    a2[rb:rb + 64, w:w + 128],
                                    cst[rb:rb + 64, rb:rb + 64], (rb, 0)))
                return out

            def emit_copy(u):
                j, a, n_s = UNITS[u]
                ps = unit_ps[u]
                src = ps[:, 0:n_s * 192].rearrange("p (s c) -> p c s", s=n_s)
                dst = tv[:, bsub:bsub + 1, :, a:a + 1, 0:n_s,
                         j:j + 1].rearrange(
                    "p one_b c one_a s one_j -> p c (one_b one_a s one_j)")
                dj = j - 2
                on_dve = (j, a) in _DVE_UNITS
                if dj == 0:
                    if on_dve:
                        nc.vector.tensor_copy(dst, src)
                    else:
                        nc.scalar.copy(dst, src)
                else:
                    m = mask_ap(dj, bl)
                    if on_dve:
                        nc.vector.tensor_scalar_mul(dst, src, m)
                    else:
                        nc.scalar.mul(dst, src, m)

            for grp in ((0, 1, 2, 3), (4, 6, 5, 7)):
                mms = [unit_mms(u) for u in grp]
                for mi in range(max(len(m) for m in mms)):
                    for m in mms:
                        if mi < len(m):
                            mm = m[mi]
                            if F32R:
                                r = mybir.dt.float32r
                                nc.tensor.transpose(
                                    mm[0].bitcast(r), mm[1].bitcast(r),
                                    mm[2].bitcast(r), tile_position=mm[3])
                            else:
                                nc.tensor.transpose(mm[0], mm[1], mm[2],
                                                    tile_position=mm[3])
                for u in grp:
                    emit_copy(u)
        nb = len(blks)
        yslice = y.ap()[blks[0] * BLK:(blks[0] + nb) * BLK, :]
        nc.sync.dma_start(
            out=yslice.rearrange("(b pp) cc -> pp b cc", b=nb),
            in_=t_[:, 0:nb * CH * NT].rearrange("p (b rest) -> p b rest", b=nb),
        )


def build_program(reps: int = 1):
    nc = bacc.Bacc(
        "TRN2",
        target_bir_lowering=False,
        debug=False,
        enable_asserts=False,
        num_devices=NB,
    )
    x = nc.dram_tensor("x", [CH, NPIX], F32, kind="ExternalInput")
    y = nc.dram_tensor("y", [NPIX, CH * NT], F32, kind="ExternalOutput")
    cst_dram = nc.inline_tensor(_const_data(), name="consts")

    with tile.TileContext(nc) as tc:
        with tc.tile_pool(name="consts", bufs=1) as cpool, \
             tc.tile_pool(name="img", bufs=1) as apool, \
             tc.tile_pool(name="store", bufs=1) as tpool, \
             tc.tile_pool(name="ps", bufs=8, space="PSUM") as pspool:

            cst = cpool.tile([128, 137], F32, tag="cst")
            nc.sync.dma_start(out=cst[:], in_=cst_dram.ap())

            # PE warmup during the load phase: ~40 zero matmuls start the
            # clock ramp (HAM) so block-0 transposes run at full rate
            wm = cpool.tile([128, 96], mybir.dt.bfloat16, tag="warm")
            nc.vector.memset(wm[:], 0.0)
            wps = pspool.tile([128, 384], F32, tag="ps")
            for _ in range(40):
                nc.tensor.matmul(wps[0:32, 0:64], wm[:, 0:32], wm[:, 32:96])

            # a1 = ch 0..127; a2 = ch 128..191 on partitions 0..63 AND 64..127
            a1 = apool.tile([128, AW], F32, tag="a1")
            a2 = apool.tile([128, AW], F32, tag="a2")
            for a in (a1, a2):
                nc.vector.memset(a[:, 0:FRONT], 0.0)
                nc.vector.memset(a[:, FRONT + NPIX:AW], 0.0)

            # store tiles: NSLOTS rotating slots of STORE_BLKS blocks each;
            # zero taps (t=14,15) written once at setup
            slots = []
            for k in range(NSLOTS):
                t_ = tpool.tile([128, STORE_BLKS * CH * NT], F32, tag=f"st{k}")
                tv = t_.rearrange("p (b c a s j) -> p b c a s j",
                                  b=STORE_BLKS, c=CH, a=2, s=2, j=4)
                nc.vector.memset(tv[:, :, :, 1:2, 1:2, 2:4], 0.0)
                slots.append(t_)

            # first two groups single-block so the store stream starts early;
            # last two single-block so the final store (critical tail after
            # the last copies) is short
            groups = [[0], [1]]
            b0 = 2
            while b0 < NBLK - 2:
                groups.append(list(range(b0, min(b0 + STORE_BLKS, NBLK - 2))))
                b0 += STORE_BLKS
            groups += [[NBLK - 2], [NBLK - 1]]

            for _rep in range(reps):
                _image_pass(nc, x, y, a1, a2, cst, slots, groups, pspool)

    nc.compile()
    return nc


_CACHE = {}


def _get_program():
    if "nc" not in _CACHE:
        _CACHE["nc"] = build_program()
    return _CACHE["nc"]


def kernel(inputs: np.ndarray) -> np.ndarray:
    x = np.ascontiguousarray(np.asarray(inputs), dtype=np.float32)
    assert x.shape == (NB, CH, HW, HW), x.shape
    nc = _get_program()
    in_maps = [
        {"x": np.ascontiguousarray(x[b].reshape(CH, NPIX))} for b in range(NB)
    ]
    res = bass_utils.run_bass_kernel_spmd(nc, in_maps, core_ids=list(range(NB)))
    out = np.stack([res.results[b]["y"] for b in range(NB)])  # (8, 2304, 3072)
    return out.reshape(NB * NPIX, CH, 4, 4)


if __name__ == "__main__":
    rng = np.random.default_rng(0)
    x = rng.standard_normal((NB, CH, HW, HW), dtype=np.float32)
    out = kernel(x)
    print(out.shape, out.dtype)


# revision 25
# speedup vs baseline: 1.0043x; 1.0043x over previous
"""Trainium2 Bass kernel for BlockSample (masked 4x4 one-hot conv / im2col).

Semantics (per batch b):
  out[(b, r, c), k, i*4+j] = x[b, k, r+i-3, c+j-2]   (zero padded)
  with taps (i=3, j>=2) forced to zero.

Full input  : (8, 192, 48, 48) f32
Full output : (8*48*48, 192, 4, 4) f32

Sharding: pure data parallel over the batch dim -> 8 NeuronCores, one
batch image per core, no cross-core communication.

Per-core plan:
  - Load image k-major into SBUF: A[ch, FRONT + r*48 + c] with zero pads
    front/back so row shifts (i-3) fall into zeros.
  - For each 128-pixel block and each live tap: PE transpose-mode matmul
    (data stationary, fp32 identity moving) -> PSUM [pix, ch].
  - DVE/ACT copy PSUM -> SBUF store tile with interleaved dest AP
    (col = ch*16 + tap), multiplying by a per-partition column-border
    mask (handles c+dj wraparound across rows).
  - Contiguous [128 pix, 12288 B] rows -> HBM stores (mostly 2-block,
    3 MB per DMA).
"""

import numpy as np

import concourse.bass as bass
import concourse.tile as tile
from concourse import bacc, mybir
from concourse import bass_utils

F32 = mybir.dt.float32

CH = 192          # channels
HW = 48           # image height/width
NPIX = HW * HW    # 2304 pixels per image
NT = 16           # taps (4x4)
NB = 8            # batch / cores
FRONT = 160       # front zero pad (>= 146 = 3*48+2)
BACK = 16         # back zero pad (>= 2)
AW = FRONT + NPIX + BACK   # padded image width in f32 elems (2480)
BLK = 128         # pixels per block
NBLK = NPIX // BLK  # 18

STORE_BLKS = 2    # blocks per store DMA (steady state)
NSLOTS = 4        # rotating store tiles
SPLIT64 = False   # split K=128 transposes into two row-group K=64 transposes
F32R = False      # run PE transposes as float32r (1.5 vs 2.0 cyc/row)

# taps: t = i*4 + j, pixel offset = (i-3)*48 + (j-2); (i=3, j>=2) are zero.
# unit (j, a) covers taps i = 2a + s for s < n_s; one PSUM tile per unit.
UNITS = []
for _j in range(4):
    for _a in range(2):
        _n = 1 if (_a == 1 and _j >= 2) else 2
        UNITS.append((_j, _a, _n))

# engine split for the PSUM->SBUF copies: DVE = 2 pairs + 2 singles, ACT = 4 pairs
_DVE_UNITS = {(0, 0), (0, 1), (2, 1), (3, 1)}


def _const_data():
    """[128, 137] = identity(128) ++ 9 border masks [dj in (-2,-1,1)] x [phase 0..2]."""
    ident = np.eye(128, dtype=np.float32)
    masks = np.ones((128, 9), dtype=np.float32)
    q = np.arange(128)
    for dji, dj in enumerate((-2, -1, 1)):
        for ph in range(3):
            phase = (BLK * ph) % HW  # 0, 32, 16
            c = (phase + q) % HW
            bad = (c + dj < 0) | (c + dj >= HW)
            masks[bad, dji * 3 + ph] = 0.0
    return np.concatenate([ident, masks], axis=1)


_MASK_COL = {-2: 0, -1: 3, 1: 6}


def _image_pass(nc, x, y, a1, a2, cst, slots, groups, pspool):
    """One full image -> output pass (loads, transposes, copies, stores)."""
    ident = cst[:, 0:128]

    def mask_ap(dj, bl):
        return cst[:, 128 + _MASK_COL[dj] + (bl % 3):129 + _MASK_COL[dj] + (bl % 3)]

    # chunked loads so block-0 compute can start early; a2 loads go on the
    # ACT HWDGE ring, a1 on the SP ring, so descriptor generation overlaps
    bounds = [0, 448, NPIX]
    for ci in range(len(bounds) - 1):
        lo, hi = bounds[ci], bounds[ci + 1]
        nc.sync.dma_start(out=a1[:, FRONT + lo:FRONT + hi],
                          in_=x.ap()[0:128, lo:hi])
        nc.scalar.dma_start(out=a2[0:64, FRONT + lo:FRONT + hi],
                            in_=x.ap()[128:192, lo:hi])

    for g, blks in enumerate(groups):
        t_ = slots[g % NSLOTS]
        tv = t_.rearrange("p (b c a s j) -> p b c a s j",
                          b=STORE_BLKS, c=CH, a=2, s=2, j=4)
        for bsub, bl in enumerate(blks):
            # One PSUM tile (= one bank) per unit. Tile serializes every
            # same-bank op pair (the P10 PE-W/DVE-R hazard guard), so a
            # unit's 4 transposes + copy form a latency-exposed chain.
            # Emit transposes interleaved across groups of 4 units: the
            # same-bank gap is >= 4 instructions (drain latency hidden)
            # and the first group's copies overlap the second group's
            # matmuls. HW-measured 200 -> 125 us/image vs naive order.
            unit_ps = {}
            for u in range(len(UNITS)):
                unit_ps[u] = pspool.tile([128, 384], F32, tag="ps",
                                         name=f"ps_{bl}_{u}")

            def unit_mms(u):
                j, a, n_s = UNITS[u]
                ps = unit_ps[u]
                out = []
                for s in range(n_s):
                    i = 2 * a + s
                    w = FRONT + bl * BLK + (i - 3) * HW + (j - 2)
                    rb = 64 * s
                    if SPLIT64:
                        # all-K=64 transposes; alternating PE row groups.
                        # NOTE: CoreSim-bit-exact but CRASHES real HW —
                        # kept only as a record, do not enable.
                        out.append((ps[:, s * 192:s * 192 + 64],
                                    a1[0:64, w:w + 128],
                                    cst[0:64, 0:64], (0, 0)))
                        out.append((ps[:, s * 192 + 64:s * 192 + 128],
                                    a1[64:128, w:w + 128],
                                    cst[64:128, 64:128], (64, 0)))
                        out.append((ps[:, s * 192 + 128:s * 192 + 192],
                                    a2[0:64, w:w + 128],
                                    cst[0:64, 0:64], (0, 0)))
                    else:
                        # K=128 chunk (ch 0..127)
                        out.append((ps[:, s * 192:s * 192 + 128],
                                    a1[:, w:w + 128], ident[0:128, 0:128],
                                    (0, 0)))
                        # K=64 chunk (ch 128..191). (A row-group-packed
                        # variant with ch duplicated on partitions 64..127
                        # measured identical — packing buys nothing.)
                        out.append((ps[:, s * 192 + 128:s * 192 + 192],
                                    a2[0:64, w:w + 128],
                                    cst[0:64, 0:64], (0, 0)))
                return out

            def emit_copy(u):
                j, a, n_s = UNITS[u]
                ps = unit_ps[u]
                src = ps[:, 0:n_s * 192].rearrange("p (s c) -> p c s", s=n_s)
                dst = tv[:, bsub:bsub + 1, :, a:a + 1, 0:n_s,
                         j:j + 1].rearrange(
                    "p one_b c one_a s one_j -> p c (one_b one_a s one_j)")
                dj = j - 2
                on_dve = (j, a) in _DVE_UNITS
                if dj == 0:
                    if on_dve:
                        nc.vector.tensor_copy(dst, src)
                    else:
                        nc.scalar.copy(dst, src)
                else:
                    m = mask_ap(dj, bl)
                    if on_dve:
                        nc.vector.tensor_scalar_mul(dst, src, m)
                    else:
                        nc.scalar.mul(dst, src, m)

            for grp in ((0, 1, 2, 3), (4, 6, 5, 7)):
                mms = [unit_mms(u) for u in grp]
                for mi in range(max(len(m) for m in mms)):
                    for m in mms:
                        if mi < len(m):
                            mm = m[mi]
                            if F32R:
                                r = mybir.dt.float32r
                                nc.tensor.transpose(
                                    mm[0].bitcast(r), mm[1].bitcast(r),
                                    mm[2].bitcast(r), tile_position=mm[3])
                            else:
                                nc.tensor.transpose(mm[0], mm[1], mm[2],
                                                    tile_position=mm[3])
                for u in grp:
                    emit_copy(u)
        nb = len(blks)
        yslice = y.ap()[blks[0] * BLK:(blks[0] + nb) * BLK, :]
        nc.sync.dma_start(
            out=yslice.rearrange("(b pp) cc -> pp b cc", b=nb),
            in_=t_[:, 0:nb * CH * NT].rearrange("p (b rest) -> p b rest", b=nb),
        )


def build_program(reps: int = 1):
    nc = bacc.Bacc(
        "TRN2",
        target_bir_lowering=False,
        debug=False,
        enable_asserts=False,
        num_devices=NB,
    )
    x = nc.dram_tensor("x", [CH, NPIX], F32, kind="ExternalInput")
    y = nc.dram_tensor("y", [NPIX, CH * NT], F32, kind="ExternalOutput")
    cst_dram = nc.inline_tensor(_const_data(), name="consts")

    with tile.TileContext(nc) as tc:
        with tc.tile_pool(name="consts", bufs=1) as cpool, \
             tc.tile_pool(name="img", bufs=1) as apool, \
             tc.tile_pool(name="store", bufs=1) as tpool, \
             tc.tile_pool(name="ps", bufs=8, space="PSUM") as pspool:

            cst = cpool.tile([128, 137], F32, tag="cst")
            nc.sync.dma_start(out=cst[:], in_=cst_dram.ap())

            # PE warmup during the load phase: ~40 zero matmuls start the
            # clock ramp (HAM) so block-0 transposes run at full rate
            wm = cpool.tile([128, 96], mybir.dt.bfloat16, tag="warm")
            nc.vector.memset(wm[:], 0.0)
            wps = pspool.tile([128, 384], F32, tag="ps")
            for _ in range(40):
                nc.tensor.matmul(wps[0:32, 0:64], wm[:, 0:32], wm[:, 32:96])

            # a1 = ch 0..127; a2 = ch 128..191 on partitions 0..63
            a1 = apool.tile([128, AW], F32, tag="a1")
            a2 = apool.tile([64, AW], F32, tag="a2")
            for a in (a1, a2):
                nc.vector.memset(a[:, 0:FRONT], 0.0)
                nc.vector.memset(a[:, FRONT + NPIX:AW], 0.0)

            # store tiles: NSLOTS rotating slots of STORE_BLKS blocks each;
            # zero taps (t=14,15) written once at setup
            slots = []
            for k in range(NSLOTS):
                t_ = tpool.tile([128, STORE_BLKS * CH * NT], F32, tag=f"st{k}")
                tv = t_.rearrange("p (b c a s j) -> p b c a s j",
                                  b=STORE_BLKS, c=CH, a=2, s=2, j=4)
                nc.vector.memset(tv[:, :, :, 1:2, 1:2, 2:4], 0.0)
                slots.append(t_)

            # first two groups single-block so the store stream starts early;
            # last two single-block so the final store (critical tail after
            # the last copies) is short
            groups = [[0], [1]]
            b0 = 2
            while b0 < NBLK - 2:
                groups.append(list(range(b0, min(b0 + STORE_BLKS, NBLK - 2))))
                b0 += STORE_BLKS
            groups += [[NBLK - 2], [NBLK - 1]]

            for _rep in range(reps):
                _image_pass(nc, x, y, a1, a2, cst, slots, groups, pspool)

    nc.compile()
    return nc


_CACHE = {}


def _get_program():
    if "nc" not in _CACHE:
        _CACHE["nc"] = build_program()
    return _CACHE["nc"]


def kernel(inputs: np.ndarray) -> np.ndarray:
    x = np.ascontiguousarray(np.asarray(inputs), dtype=np.float32)
    assert x.shape == (NB, CH, HW, HW), x.shape
    nc = _get_program()
    in_maps = [
        {"x": np.ascontiguousarray(x[b].reshape(CH, NPIX))} for b in range(NB)
    ]
    res = bass_utils.run_bass_kernel_spmd(nc, in_maps, core_ids=list(range(NB)))
    out = np.stack([res.results[b]["y"] for b in range(NB)])  # (8, 2304, 3072)
    return out.reshape(NB * NPIX, CH, 4, 4)


if __name__ == "__main__":
    rng = np.random.default_rng(0)
    x = rng.standard_normal((NB, CH, HW, HW), dtype=np.float32)
    out = kernel(x)
    print(out.shape, out.dtype)
